# revision 21
# baseline (speedup 1.0000x reference)
"""Varlen causal attention (flash_attn_varlen semantics) on 8 Trainium2 cores.

Sharding: 16 heads across 8 cores (2 heads/core, Ulysses-style head shard,
identity comms).  Each core runs the same SPMD Bass program on its head slice.

v2 design — all data prep on the HOST, device does only the attention math:
  host:  per head, Q^T and K^T are pre-transposed to [D, L] (bf16, or fp8e4
         d-pair-interleaved [64, 2, L] for DoubleRow), V is packed to
         [128, nb, 130] bf16 with a leading ones column (softmax denominator
         falls out of the PV matmul), and the exact {0,1} block masks
         (causal x segment, sliced per in-mask 128x128 unit, column-trimmed
         to the valid prefix) are baked into one bf16 strip shared by all
         heads.  Inputs are uploaded in exactly the on-chip layout, halving
         HBM traffic vs f32 and eliminating every on-device transpose/cast.
  device per head:
         S^T tile = K_j @ Q^T per in-mask unit (bf16, or fp8 DoubleRow at
         2x PE rate), batched exp on ScalarE straight from PSUM (bf16 out),
         masked units get one DVE multiply per contiguous cluster against
         the host mask strip, then PV matmuls (P^T chunk stationary, V+ones
         moving) accumulate O[q, 0:130] per chunk.  Finalize: one DVE
         PSUM->SBUF copy per chunk pair, then GpSimd normalize_recip
         divides by the denominator and casts to bf16 (keeps DVE/ACT free).
         Output is written bf16 and upcast on the host.
Column trimming: units whose q-columns die at a segment boundary are
computed/exp'd only up to the exact boundary column; their PV matmuls
write only the live output partitions (full-width unit runs first so
start=True initializes the whole PSUM region).
"""

import numpy as np
import ml_dtypes

L = 4096
H = 16
D = 128
N_CORES = 8
HPC = H // N_CORES
QB = 128
NB = L // QB
SCALE = 1.0 / float(np.sqrt(D))
SPANS = [2] * 8 + [4] * 3 + [2, 1, 1]  # q blocks per superblock (sums to 32)
# Schraudolph exp for offloaded groups: bf16 = bitcast(int16(x*A + B)),
# 2^x via the exponent bits; C=7 splits the difference between round and
# floor conversion (~1.8% rms, ~4% max per-prob error on those groups)
EXP_A = SCALE * 128 * 1.4426950408889634
EXP_B = 127.0 * 128 - 7.0
GROUP_COLS = 1024             # S^T PSUM group tile width (f32, 2 banks)
BANK_COLS = 512               # PSUM bank width in f32 cols
USE_FP8 = False               # fp8e4 DoubleRow for the S matmul


def _plan(cu: np.ndarray):
    """Host-side specialization: block-sparse unit list (column-trimmed),
    group packing, run merging, mask strip, PV lists."""
    cu = np.asarray(cu, dtype=np.int64)
    tok = np.arange(L)
    seg = np.searchsorted(cu[1:-1], tok, side="right")

    def sub(j, i):
        # S^T layout: rows k (partition), cols q
        sq = seg[i * QB:(i + 1) * QB]
        sk = seg[j * QB:(j + 1) * QB]
        same = sk[:, None] == sq[None, :]
        causal = (j * QB + np.arange(QB))[:, None] <= \
                 (i * QB + np.arange(QB))[None, :]
        return same & causal

    plan = []
    strips = []
    strip_off = 0
    i0 = 0
    for span in SPANS:
        i1 = i0 + span - 1
        units = []
        for i in range(i0, i1 + 1):
            for j in range(0, i + 1):
                m = sub(j, i)
                vc = m.any(axis=0)
                w = int(vc.sum())
                if w == 0:
                    continue
                assert vc[:w].all() and not vc[w:].any(), (j, i)
                units.append({"j": j, "i": i, "w": w,
                              "needs": not bool(m[:, :w].all()),
                              "m": m[:, :w]})
        # masked units first (clusters them for one DVE mul), wide first
        units.sort(key=lambda u: (not u["needs"], -u["w"], u["j"], u["i"]))

        groups = []
        cur = GROUP_COLS  # force new group
        for u in units:
            w = u["w"]
            # advance past a bank boundary the unit would straddle
            nxt = cur
            if (nxt % BANK_COLS) + w > BANK_COLS:
                nxt = (nxt // BANK_COLS + 1) * BANK_COLS
            if nxt + w > GROUP_COLS:
                groups.append({"units": [], "used": 0})
                nxt = 0
            u["g"] = len(groups) - 1
            u["off"] = nxt
            groups[-1]["units"].append(u)
            cur = nxt + w
            groups[-1]["used"] = cur

        sb_m0 = strip_off
        for g in groups:
            ordered = sorted(g["units"], key=lambda u: u["off"])
            runs = []
            for u in ordered:
                r = runs[-1] if runs else None
                if (r is not None and u["w"] == QB and r["w"] == QB
                        and r["j"] == u["j"] and r["i"] + r["n"] == u["i"]
                        and r["off"] + r["n"] * QB == u["off"]
                        and r["n"] < 4
                        and r["off"] // BANK_COLS ==
                            (u["off"] + QB - 1) // BANK_COLS):
                    r["n"] += 1
                else:
                    runs.append({"off": u["off"], "j": u["j"], "i": u["i"],
                                 "n": 1, "w": u["w"]})
            g["runs"] = runs
            mops = []
            for u in ordered:
                if not u["needs"]:
                    continue
                strips.append(u["m"].astype(np.float32))
                if mops and mops[-1]["off"] + mops[-1]["w"] == u["off"]:
                    mops[-1]["w"] += u["w"]
                else:
                    mops.append({"off": u["off"], "w": u["w"],
                                 "soff": strip_off})
                strip_off += u["w"]
            g["mops"] = mops

        pv = {i: [] for i in range(i0, i1 + 1)}
        for gi, g in enumerate(groups):
            for u in g["units"]:
                pv[u["i"]].append((gi, u["off"], u["w"], u["j"]))
        for i in pv:
            # widest first: the first matmul (start=True) must cover all
            # 128 output partitions; trimmed units accumulate a prefix
            pv[i].sort(key=lambda t: (-t[2], t[3]))
            assert pv[i][0][2] == QB, (i, pv[i])
        plan.append({"groups": groups, "pv": pv, "i0": i0, "n": span,
                     "m0": sb_m0, "m1": strip_off})
        i0 += span

    mstrip = (np.concatenate(strips, axis=1) if strips
              else np.zeros((QB, 1), np.float32))
    return plan, mstrip


def _build(cu: np.ndarray, mask_w: int):
    import concourse.mybir as mybir
    import concourse.tile as tile
    from concourse import bacc
    from concourse.masks import make_identity

    f32 = mybir.dt.float32
    bf16 = mybir.dt.bfloat16
    i16 = mybir.dt.int16
    fp8 = mybir.dt.float8e4
    AF = mybir.ActivationFunctionType
    ALU = mybir.AluOpType
    DR = mybir.MatmulPerfMode.DoubleRow
    plan, _ = _plan(cu)
    # exp engine per (superblock, group).  Offloading exp to DVE lengthens
    # the S->exp->PV chain and starves PE (measured +6us), so everything
    # stays on ACT; the Schraudolph path below is kept for experiments.
    offl = {}

    nc = bacc.Bacc("TRN2", target_bir_lowering=False, debug=False,
                   num_devices=N_CORES)
    if USE_FP8:
        q_d = nc.dram_tensor("qT", [64, HPC, 2, L], fp8, kind="ExternalInput")
        k_d = nc.dram_tensor("kT", [64, HPC, 2, L], fp8, kind="ExternalInput")
    else:
        q_d = nc.dram_tensor("qT", [128, HPC, L], bf16, kind="ExternalInput")
        k_d = nc.dram_tensor("kT", [128, HPC, L], bf16, kind="ExternalInput")
    v_d = nc.dram_tensor("vA", [128, HPC, NB, 130], bf16,
                         kind="ExternalInput")
    m_d = nc.dram_tensor("msk", [128, mask_w], bf16, kind="ExternalInput")
    o_d = nc.dram_tensor("out", [128, HPC, NB, D], bf16,
                         kind="ExternalOutput")

    with tile.TileContext(nc) as tc:
        with (
            tc.tile_pool(name="consts", bufs=1) as consts,
            tc.tile_pool(name="stage", bufs=1) as stage,
            tc.tile_pool(name="psb", bufs=20) as psb,
            tc.tile_pool(name="osb", bufs=3) as osb,
            tc.tile_pool(name="rsb", bufs=4) as rsb,
            tc.tile_pool(name="s_ps", bufs=3, space="PSUM") as s_ps,
            tc.tile_pool(name="o_ps", bufs=2, space="PSUM") as o_ps,
        ):
            ident = consts.tile([128, 128], f32)
            make_identity(nc, ident[:])
            # preload the Exp table (~1.4us) during the initial DMA wait
            warm = consts.tile([128, 1], f32)
            nc.scalar.activation(warm[:], ident[:, 0:1], AF.Exp)
            # keep PE busy early so its clock ramps before the real work
            for _ in range(3):
                wps = s_ps.tile([128, 1024], f32, tag="s", name="wps")
                for t in range(4):
                    nc.tensor.transpose(wps[:, t * 128:(t + 1) * 128],
                                        ident[:], ident[:])

            # ---- input DMA: device-layout tensors, early tokens first ----
            if USE_FP8:
                qs = stage.tile([64, HPC, 2, L], fp8, tag="qs")
                ks = stage.tile([64, HPC, 2, L], fp8, tag="ks")
            else:
                qs = stage.tile([128, HPC, L], bf16, tag="qs")
                ks = stage.tile([128, HPC, L], bf16, tag="ks")
            vs = stage.tile([128, HPC, NB, 130], bf16, tag="vs")
            msk = stage.tile([128, mask_w], bf16, tag="msk")
            spans = [(0, 2), (2, 2), (4, 4), (8, 4), (12, 4)] + \
                    [(b, 8) for b in range(16, NB, 8)]
            m_sent = 0
            for b0, nb in spans:
                r = slice(b0 * QB, (b0 + nb) * QB)
                if USE_FP8:
                    nc.sync.dma_start(ks[:, :, :, r], k_d[:, :, :, r])
                    nc.gpsimd.dma_start(qs[:, :, :, r], q_d[:, :, :, r])
                else:
                    nc.sync.dma_start(ks[:, :, r], k_d[:, :, r])
                    nc.gpsimd.dma_start(qs[:, :, r], q_d[:, :, r])
                nc.gpsimd.dma_start(vs[:, :, b0:b0 + nb, :],
                                    v_d[:, :, b0:b0 + nb, :])
                # mask segments for superblocks fully covered by this span
                m_need = max((sb["m1"] for sb in plan
                              if sb["i0"] + sb["n"] <= b0 + nb),
                             default=0)
                if m_need > m_sent:
                    nc.sync.dma_start(msk[:, m_sent:m_need],
                                      m_d[:, m_sent:m_need])
                    m_sent = m_need
            if m_sent < mask_w:
                nc.sync.dma_start(msk[:, m_sent:], m_d[:, m_sent:])

            hstate = [{"pvq": [], "o_open": {}, "eager": False,
                       "ost": {"tile": None, "i0": 0, "filled": 0}}
                      for _ in range(HPC)]

            def flush_out(h):
                st = hstate[h]["ost"]
                nf = st["filled"]
                if not nf:
                    return
                nc.sync.dma_start(o_d[:, h, st["i0"]:st["i0"] + nf, :],
                                  st["tile"][:, 0:nf, :])
                st["tile"] = None
                st["filled"] = 0

            def make_pv_queue(h, I, ptiles):
                sbp = plan[I]
                i0 = sbp["i0"]
                nch = sbp["n"]
                q = []
                for p0 in range(0, nch, 2):
                    np_ = min(2, nch - p0)
                    pair = p0 // 2
                    for c in range(p0, p0 + np_):
                        lst = sbp["pv"][i0 + c]
                        for nn, (g, off, w, j) in enumerate(lst):
                            q.append(("mm", pair, (c - p0) * 130, ptiles[g],
                                      off, w, j, nn == 0, nn == len(lst) - 1))
                    q.append(("fin", pair, i0, p0, np_))
                return q

            def pump_pv(h, k, steal=False):
                # drain this head's queue; small pumps may steal from the
                # other head so PE always has PV filler between S matmuls
                done = pump_one(h, k)
                if steal and done < k and HPC > 1:
                    pump_one(1 - h, k - done)

            def pump_one(h, k):
                hs = hstate[h]
                q = hs["pvq"]
                st = hs["ost"]
                done = 0
                while q and done < k:
                    it = q.pop(0)
                    if it[0] == "mm":
                        _, pair, slot, p_t, off, w, j, s0, s1 = it
                        ot = hs["o_open"].get(pair)
                        if ot is None:
                            ot = o_ps.tile([128, 512], f32, tag="o",
                                           name="o_t")
                            hs["o_open"][pair] = ot
                        nc.tensor.matmul(
                            ot[0:w, slot:slot + 130],
                            p_t[:, off:off + w],
                            vs[:, h, j, 0:130], start=s0, stop=s1)
                        done += 1
                    else:
                        _, pair, i0, p0, np_ = it
                        ot = hs["o_open"].pop(pair)
                        ob = rsb.tile([128, 2, 130], f32, tag="r")
                        nc.vector.tensor_copy(
                            ob[:, 0:np_, :],
                            ot[:, 0:np_ * 130].rearrange(
                                "p (c x) -> p c x", c=np_))
                        if st["tile"] is None:
                            st["tile"] = osb.tile([128, 4, 128], bf16,
                                                  tag=f"ost{h}", name="ost")
                            st["i0"] = i0 + p0
                        for c in range(np_):
                            nc.gpsimd.normalize_recip(
                                st["tile"][:, st["filled"] + c, :],
                                ob[:, c, 1:129], ob[:, c, 0:1])
                        st["filled"] += np_
                        if st["filled"] >= (2 if hs["eager"] else 4):
                            flush_out(h)
                return done

            def s_matmul(s_t, off, j, i, n, w, h):
                cols = slice(i * QB, i * QB + (n - 1) * QB + w)
                if USE_FP8:
                    nc.tensor.matmul(
                        s_t[:, off:off + (n - 1) * QB + w],
                        ks[:, h, :, j * QB:(j + 1) * QB],
                        qs[:, h, :, cols], start=True, stop=True,
                        perf_mode=DR)
                else:
                    nc.tensor.matmul(
                        s_t[:, off:off + (n - 1) * QB + w],
                        ks[:, h, j * QB:(j + 1) * QB],
                        qs[:, h, cols], start=True, stop=True)

            def emit_groups(h, I):
                sbp = plan[I]
                ptiles = []
                for gi, g in enumerate(sbp["groups"]):
                    s_t = s_ps.tile([128, 1024], f32, tag="s")
                    p_t = psb.tile([128, 1024], bf16, tag="p")
                    for r in g["runs"]:
                        s_matmul(s_t, r["off"], r["j"], r["i"], r["n"],
                                 r["w"], h)
                        pump_pv(h, 3, steal=True)
                    used = g["used"]
                    kind = offl.get((I, gi), "act")
                    if kind == "act":
                        nc.scalar.activation(p_t[:, 0:used], s_t[:, 0:used],
                                             AF.Exp, scale=SCALE)
                    else:
                        eng = nc.vector if kind == "dve" else nc.gpsimd
                        eng.tensor_scalar(p_t[:, 0:used].bitcast(i16),
                                          s_t[:, 0:used], EXP_A, EXP_B,
                                          op0=ALU.mult, op1=ALU.add)
                    for m in g["mops"]:
                        sl = p_t[:, m["off"]:m["off"] + m["w"]]
                        nc.vector.tensor_mul(
                            sl, sl, msk[:, m["soff"]:m["soff"] + m["w"]])
                    ptiles.append(p_t)
                return ptiles

            # ---- main loop: heads interleaved at superblock granularity,
            # one superblock deep software pipeline (superblock I's S
            # matmuls pump the I-1 PV queue).  Last head's two smallest
            # superblocks run last so the final drain is short.
            pairs = [(I, h) for I in range(len(plan)) for h in range(HPC)]
            lh = HPC - 1
            if len(plan) > 4 and HPC > 1:
                for p in ((0, lh), (1, lh)):
                    pairs.remove(p)
                    pairs.append(p)
            last_idx = {h: max(n for n, (_, hh) in enumerate(pairs)
                               if hh == h) for h in range(HPC)}
            for n, (I, h) in enumerate(pairs):
                ptiles = emit_groups(h, I)
                pump_pv(h, 1 << 30)
                hstate[h]["pvq"] = make_pv_queue(h, I, ptiles)
                for hh in range(HPC):
                    if hstate[hh]["pvq"] and n >= last_idx[hh] + 1:
                        hstate[hh]["eager"] = True
                        pump_pv(hh, 1 << 30)
                        flush_out(hh)
            for h in range(HPC):
                hstate[h]["eager"] = True
                pump_pv(h, 1 << 30)
                flush_out(h)

    nc.compile()
    return nc


def _run(query, key, value, cu_seqlens, trace=False, **spmd_kwargs):
    from concourse import bass_utils

    query = np.asarray(query, dtype=np.float32)
    key = np.asarray(key, dtype=np.float32)
    value = np.asarray(value, dtype=np.float32)
    cu = np.asarray(cu_seqlens, dtype=np.int64)

    _, mstrip = _plan(cu)
    nc = _build(cu, mstrip.shape[1])

    bfloat16 = ml_dtypes.bfloat16
    f8 = ml_dtypes.float8_e4m3
    msk_np = np.ascontiguousarray(mstrip.astype(bfloat16))
    in_maps = []
    for c in range(N_CORES):
        hs = slice(c * HPC, (c + 1) * HPC)
        q_c = query[:, hs, :]          # [L, HPC, D]
        k_c = key[:, hs, :]
        v_c = value[:, hs, :]
        if USE_FP8:
            # [64, HPC, 2, L]: partition p holds d=p (k-tile 0) / d=64+p
            qT = np.ascontiguousarray(
                q_c.reshape(L, HPC, 2, 64).transpose(3, 1, 2, 0)
                .astype(f8))
            kT = np.ascontiguousarray(
                k_c.reshape(L, HPC, 2, 64).transpose(3, 1, 2, 0)
                .astype(f8))
        else:
            qT = np.ascontiguousarray(
                q_c.transpose(2, 1, 0).astype(bfloat16))   # [D, HPC, L]
            kT = np.ascontiguousarray(
                k_c.transpose(2, 1, 0).astype(bfloat16))
        vA = np.ones((128, HPC, NB, 130), dtype=np.float32)
        vA[:, :, :, 1:129] = v_c.reshape(NB, QB, HPC, D).transpose(1, 2, 0, 3)
        in_maps.append({"qT": qT, "kT": kT,
                        "vA": np.ascontiguousarray(vA.astype(bfloat16)),
                        "msk": msk_np})
    res = bass_utils.run_bass_kernel_spmd(nc, in_maps, list(range(N_CORES)),
                                          trace=trace, **spmd_kwargs)
    out = np.empty((L, H, D), dtype=np.float32)
    for c in range(N_CORES):
        o = np.asarray(res.results[c]["out"])         # [128, HPC, NB, D] bf16
        out[:, c * HPC:(c + 1) * HPC, :] = (
            o.transpose(2, 0, 1, 3).reshape(L, HPC, D).astype(np.float32))
    return out, res


def kernel(query, key, value, cu_seqlens):
    out, _ = _run(query, key, value, cu_seqlens)
    return out
